# revision 6
# baseline (speedup 1.0000x reference)
"""DLRM (embedding gather + bottom MLP + pairwise interactions + top MLP)
on 8 Trainium2 NeuronCores, data-parallel over the batch.

Sharding: batch 16384 -> 8 cores x 2048 samples. The embedding table
(26 x 100000 x 128 fp32, 1.33 GB) is replicated to every core's HBM, so no
collectives are needed; each core runs the full model on its batch slice.

Per-core device program:
  - indirect_dma_start gathers 26 rows/sample (512B descriptors) into
    [128 samples, 26*128] SBUF tiles, one DMA per 128-sample block.
  - DVE casts to bf16; PE transposes each [128,128] block to build
    ZT [E=128, 27 cols/sample] (26 features + bottom-MLP output).
  - Interactions: per-4-sample packed matmuls lhsT=rhs=ZT-slice
    [128, 4x27] -> S [108,108] PSUM (block-diagonal useful entries).
  - The strict-upper-triangle extraction is folded into the top-MLP first
    layer: W1full[27i+j] = 0.5*Wt1[pair(i,j)] (0 on diagonal), so layer 1
    consumes all 729 S values. S is routed through a DRAM round-trip
    ([sample, 729] rows) + PE transposes to get X1T [729, batch].
  - Top MLP 729(+128 dense)->1024->1024->512->256->1 in bf16 with fp32
    PSUM accumulation, transposed activations, sigmoid output.
"""

import numpy as np
import ml_dtypes

import concourse.bass as bass
import concourse.mybir as mybir
import concourse.tile as tile
from concourse import bacc
from concourse.bass import ds, ts

F32 = mybir.dt.float32
BF16 = mybir.dt.bfloat16
I32 = mybir.dt.int32

N_CAT = 26
N_FEAT = 27          # 26 embeddings + dense
E = 128
P = 128
N_CORES = 8

AFT = mybir.ActivationFunctionType


class Cfg:
    def __init__(self, vocab=100000, nblk=16):
        self.vocab = vocab
        self.nblk = nblk                  # 128-sample blocks per core
        self.bc = nblk * P                # batch per core
        self.vblk = min(4, nblk)          # blocks per btile
        self.vt = self.vblk * P           # btile size (free dim N)
        self.nv = nblk // self.vblk       # btiles per core


def build_kernel(nc: bass.Bass, tc: tile.TileContext, cfg: Cfg):
    from contextlib import ExitStack
    with ExitStack() as ctx:
        _build_kernel(nc, tc, cfg, ctx)


def _build_kernel(nc: bass.Bass, tc: tile.TileContext, cfg: Cfg, ctx):
    NBLK, BC, VT, NV, VBLK = cfg.nblk, cfg.bc, cfg.vt, cfg.nv, cfg.vblk

    # ---------------- DRAM I/O ----------------
    emb = nc.dram_tensor("emb", [N_CAT * cfg.vocab, E], F32, kind="ExternalInput").ap()
    idx_d = nc.dram_tensor("idx", [P, NBLK * N_CAT], I32, kind="ExternalInput").ap()
    xt_d = nc.dram_tensor("xt", [13, BC], BF16, kind="ExternalInput").ap()

    def win(name, shape):
        return nc.dram_tensor(name, shape, BF16, kind="ExternalInput").ap()

    wd1_d = win("wd1", [13, 512])
    wd2_d = win("wd2", [512, 256])
    wdf_d = win("wdf", [256, 128])
    w1f_d = win("w1f", [729, 1024])
    w1d_d = win("w1d", [128, 1024])
    wt2_d = win("wt2", [1024, 1024])
    wt3_d = win("wt3", [1024, 512])
    wt4_d = win("wt4", [512, 256])
    wo_d = win("wo", [256, 1])

    def bin_(name, shape):
        return nc.dram_tensor(name, shape, F32, kind="ExternalInput").ap()

    bd1_d = bin_("bd1", [P, 4])
    bd2_d = bin_("bd2", [P, 2])
    bdf_d = bin_("bdf", [P, 1])
    bt1_d = bin_("bt1", [P, 8])
    bt2_d = bin_("bt2", [P, 8])
    bt3_d = bin_("bt3", [P, 4])
    bt4_d = bin_("bt4", [P, 2])
    bo_d = bin_("bo", [1, 1])
    ident_d = nc.dram_tensor("ident", [P, P], BF16, kind="ExternalInput").ap()

    out_d = nc.dram_tensor("out", [BC, 1], F32, kind="ExternalOutput").ap()

    # DRAM scratch for the S -> X1 relayout, one per btile so phase II of
    # btile v only depends on btile v's writes.
    x1d = [
        nc.dram_tensor(f"x1scratch{v}", [VT, 729], BF16, kind="Internal").ap()
        for v in range(NV)
    ]

    # ---------------- pools ----------------
    const = ctx.enter_context(tc.tile_pool(name="const", bufs=1))
    gpool = ctx.enter_context(tc.tile_pool(name="gather", bufs=2))
    zpool = ctx.enter_context(tc.tile_pool(name="zt", bufs=2))
    spool = ctx.enter_context(tc.tile_pool(name="spack", bufs=2))
    x1pool = ctx.enter_context(tc.tile_pool(name="x1", bufs=2))
    xtpool = ctx.enter_context(tc.tile_pool(name="x1t", bufs=2))
    opool = ctx.enter_context(tc.tile_pool(name="acts", bufs=1))
    p_t = ctx.enter_context(tc.tile_pool(name="ptrans", bufs=2, space="PSUM"))
    p_s = ctx.enter_context(tc.tile_pool(name="pint", bufs=2, space="PSUM"))
    p_m = ctx.enter_context(tc.tile_pool(name="pmlp", bufs=3, space="PSUM"))

    # ---------------- load constants ----------------
    idx_sb = const.tile([P, NBLK * N_CAT], I32)
    nc.sync.dma_start(out=idx_sb[:], in_=idx_d)
    ident = const.tile([P, P], BF16)
    nc.sync.dma_start(out=ident[:], in_=ident_d)

    def load_w(name, d, k, n):
        """Load [K, N] weight as list of [<=128, n] SBUF tiles."""
        tiles = []
        for i in range((k + P - 1) // P):
            ck = min(P, k - i * P)
            t = const.tile([ck, n], BF16, name=f"{name}_{i}")
            nc.sync.dma_start(out=t[:], in_=d[i * P : i * P + ck, :])
            tiles.append(t)
        return tiles

    wd1 = load_w("wd1", wd1_d, 13, 512)
    wd2 = load_w("wd2", wd2_d, 512, 256)
    wdf = load_w("wdf", wdf_d, 256, 128)
    w1f = load_w("w1f", w1f_d, 729, 1024)
    w1d = load_w("w1d", w1d_d, 128, 1024)
    wt2 = load_w("wt2", wt2_d, 1024, 1024)
    wt3 = load_w("wt3", wt3_d, 1024, 512)
    wt4 = load_w("wt4", wt4_d, 512, 256)
    wo = load_w("wo", wo_d, 256, 1)

    def load_b(name, d, nm):
        t = const.tile([d.shape[0], nm], F32, name=name)
        nc.sync.dma_start(out=t[:], in_=d)
        return t

    bd1 = load_b("bd1", bd1_d, 4)
    bd2 = load_b("bd2", bd2_d, 2)
    bdf = load_b("bdf", bdf_d, 1)
    bt1 = load_b("bt1", bt1_d, 8)
    bt2 = load_b("bt2", bt2_d, 8)
    bt3 = load_b("bt3", bt3_d, 4)
    bt4 = load_b("bt4", bt4_d, 2)
    bo = load_b("bo", bo_d, 1)

    # ---------------- bottom MLP (whole core batch) ----------------
    with tc.tile_pool(name="bottom", bufs=1) as bot:
        xtb = bot.tile([13, BC], BF16)
        nc.sync.dma_start(out=xtb[:], in_=xt_d)

        h1 = [bot.tile([P, BC], BF16, name=f"h1_{m}") for m in range(4)]
        for m in range(4):
            for v in range(NV):
                pm = p_m.tile([P, VT], F32)
                nc.tensor.matmul(
                    pm[:], wd1[0][:, ts(m, P)], xtb[:, ts(v, VT)],
                    start=True, stop=True,
                )
                nc.scalar.activation(
                    h1[m][:, ts(v, VT)], pm[:], AFT.Relu, bias=bd1[:, m : m + 1]
                )

        h2 = [bot.tile([P, BC], BF16, name=f"h2_{m}") for m in range(2)]
        for m in range(2):
            for v in range(NV):
                pm = p_m.tile([P, VT], F32)
                for k in range(4):
                    nc.tensor.matmul(
                        pm[:], wd2[k][:, ts(m, P)], h1[k][:, ts(v, VT)],
                        start=(k == 0), stop=(k == 3),
                    )
                nc.scalar.activation(
                    h2[m][:, ts(v, VT)], pm[:], AFT.Relu, bias=bd2[:, m : m + 1]
                )

        denseT = const.tile([P, BC], BF16)
        for v in range(NV):
            pm = p_m.tile([P, VT], F32)
            for k in range(2):
                nc.tensor.matmul(
                    pm[:], wdf[k][:], h2[k][:, ts(v, VT)],
                    start=(k == 0), stop=(k == 1),
                )
            nc.scalar.activation(
                denseT[:, ts(v, VT)], pm[:], AFT.Relu, bias=bdf[:, 0:1]
            )

    # ---------------- per-btile main loop ----------------
    for v in range(NV):
        # ---- phase I: gather + transpose + interactions for VBLK blocks ----
        for tt in range(VBLK):
            t = v * VBLK + tt
            g = gpool.tile([P, N_CAT * E], F32)
            nc.gpsimd.indirect_dma_start(
                out=g[:],
                out_offset=None,
                in_=emb,
                in_offset=bass.IndirectOffsetOnAxis(
                    ap=idx_sb[:, t * N_CAT : (t + 1) * N_CAT], axis=0
                ),
            )
            gb = gpool.tile([P, N_CAT * E], BF16)
            nc.vector.tensor_copy(out=gb[:], in_=g[:])

            # ZT: [E, (s, j)-major]; col s*27+j so 4-sample groups are
            # contiguous 108-col slices (walrus needs 1 free dim on rhs)
            zt = zpool.tile([P, P * N_FEAT], BF16)
            ztv = zt[:].rearrange("p (s j) -> p s j", j=N_FEAT)
            for j in range(N_CAT):
                pt = p_t.tile([P, P], BF16)
                nc.tensor.transpose(pt[:], gb[:, ts(j, P)], ident[:])
                nc.any.tensor_copy(out=ztv[:, :, j], in_=pt[:])
            nc.vector.tensor_copy(
                out=ztv[:, :, N_CAT], in_=denseT[:, ts(t, P)]
            )

            # interactions: 32 packed matmuls of 4 samples each
            spk = spool.tile([108, 32 * 108], BF16)
            for q in range(32):
                pi = p_s.tile([108, 108], F32)
                op = zt[:, ds(4 * q * N_FEAT, 108)]
                nc.tensor.matmul(pi[:], op, op, start=True, stop=True)
                nc.any.tensor_copy(out=spk[:, ts(q, 108)], in_=pi[:])

            # write S -> x1 scratch rows [sample, 729]
            # spk row (s1*27+j1), col (q*108 + s2*27 + j2); take s1==s2==s
            for s in range(4):
                src = spk[ds(s * 27, 27), :].rearrange(
                    "p (q s2 j2) -> p q s2 j2", s2=4, j2=27
                )[:, :, s, :]
                # dest row b = tt*128 + 4q + s ; iterate (j1, q, j2)
                dstv = x1d[v].rearrange(
                    "(tt q s) (j1 j2) -> tt s j1 q j2", tt=VBLK, s=4, j2=27
                )[tt, s, :, :, :]
                nc.sync.dma_start(out=dstv, in_=src)

        # ---- phase II: relayout X1 -> X1T and top MLP ----
        x1t = [xtpool.tile([min(P, 729 - k * P), VT], BF16, name=f"x1t_{k}")
               for k in range(6)]
        for tt in range(VBLK):
            x1s = x1pool.tile([P, 729], BF16)
            nc.sync.dma_start(out=x1s[:], in_=x1d[v][ts(tt, P), :])
            for k in range(6):
                ck = min(P, 729 - k * P)
                pt = p_t.tile([P, P], BF16)
                nc.tensor.transpose(pt[:ck, :], x1s[:, ds(k * P, ck)], ident[:])
                nc.any.tensor_copy(out=x1t[k][:, ts(tt, P)], in_=pt[:ck, :])

        # layer 1: K = 729 pairs + 128 dense
        o1 = opool.tile([P, 8 * VT], BF16, name="o1")
        for m in range(8):
            pm = p_m.tile([P, VT], F32)
            for k in range(6):
                nc.tensor.matmul(
                    pm[:], w1f[k][:, ts(m, P)], x1t[k][:],
                    start=(k == 0), stop=False,
                )
            nc.tensor.matmul(
                pm[:], w1d[0][:, ts(m, P)], denseT[:, ts(v, VT)],
                start=False, stop=True,
            )
            nc.scalar.activation(
                o1[:, ts(m, VT)], pm[:], AFT.Relu, bias=bt1[:, m : m + 1]
            )

        o2 = opool.tile([P, 8 * VT], BF16, name="o2")
        for m in range(8):
            pm = p_m.tile([P, VT], F32)
            for k in range(8):
                nc.tensor.matmul(
                    pm[:], wt2[k][:, ts(m, P)], o1[:, ts(k, VT)],
                    start=(k == 0), stop=(k == 7),
                )
            nc.scalar.activation(
                o2[:, ts(m, VT)], pm[:], AFT.Relu, bias=bt2[:, m : m + 1]
            )

        o3 = opool.tile([P, 4 * VT], BF16, name="o3")
        for m in range(4):
            pm = p_m.tile([P, VT], F32)
            for k in range(8):
                nc.tensor.matmul(
                    pm[:], wt3[k][:, ts(m, P)], o2[:, ts(k, VT)],
                    start=(k == 0), stop=(k == 7),
                )
            nc.scalar.activation(
                o3[:, ts(m, VT)], pm[:], AFT.Relu, bias=bt3[:, m : m + 1]
            )

        o4 = opool.tile([P, 2 * VT], BF16, name="o4")
        for m in range(2):
            pm = p_m.tile([P, VT], F32)
            for k in range(4):
                nc.tensor.matmul(
                    pm[:], wt4[k][:, ts(m, P)], o3[:, ts(k, VT)],
                    start=(k == 0), stop=(k == 3),
                )
            nc.scalar.activation(
                o4[:, ts(m, VT)], pm[:], AFT.Relu, bias=bt4[:, m : m + 1]
            )

        pm = p_m.tile([1, VT], F32)
        for k in range(2):
            nc.tensor.matmul(
                pm[:], wo[k][:], o4[:, ts(k, VT)],
                start=(k == 0), stop=(k == 1),
            )
        zf = x1pool.tile([1, VT], F32, name="zfinal")
        nc.scalar.activation(zf[:], pm[:], AFT.Sigmoid, bias=bo[:])
        nc.sync.dma_start(
            out=out_d[ts(v, VT), :].rearrange("b one -> one b"), in_=zf[:]
        )


# ---------------------------------------------------------------------------
# host side
# ---------------------------------------------------------------------------

_CACHE = {}


def _get_nc(cfg: Cfg):
    key = (cfg.vocab, cfg.nblk)
    if key in _CACHE:
        return _CACHE[key]
    nc = bacc.Bacc(
        "TRN2",
        target_bir_lowering=False,
        debug=False,
        enable_asserts=False,
        num_devices=N_CORES,
    )
    with tile.TileContext(nc) as tc:
        build_kernel(nc, tc, cfg)
    nc.compile()
    _CACHE[key] = nc
    return nc


def _prep_host(inputs, cfg: Cfg):
    """Build the per-core in_maps from full inputs."""
    bf = ml_dtypes.bfloat16
    emb = np.ascontiguousarray(
        inputs["emb_table"].reshape(N_CAT * cfg.vocab, E), dtype=np.float32
    )
    cat = np.asarray(inputs["cat_idx"])
    dx = np.asarray(inputs["dense_x"], dtype=np.float32)

    iu, ju = np.triu_indices(N_FEAT, k=1)
    wt1 = np.asarray(inputs["Wt1"], dtype=np.float32)  # [479, 1024]
    w1f = np.zeros((N_FEAT, N_FEAT, 1024), dtype=np.float32)
    w1f[iu, ju] = 0.5 * wt1[: len(iu)]
    w1f[ju, iu] = 0.5 * wt1[: len(iu)]
    w1f = w1f.reshape(729, 1024)
    w1d = wt1[len(iu) :]  # [128, 1024]

    def b2(x, nm):  # bias [N] -> [128, nm]
        return np.ascontiguousarray(
            np.asarray(x, np.float32).reshape(nm, P).T
        )

    shared = dict(
        emb=emb,
        wd1=np.asarray(inputs["Wd1"], bf),
        wd2=np.asarray(inputs["Wd2"], bf),
        wdf=np.asarray(inputs["Wdf"], bf),
        w1f=np.asarray(w1f, bf),
        w1d=np.ascontiguousarray(np.asarray(w1d, bf)),
        wt2=np.asarray(inputs["Wt2"], bf),
        wt3=np.asarray(inputs["Wt3"], bf),
        wt4=np.asarray(inputs["Wt4"], bf),
        wo=np.asarray(inputs["Wo"], bf),
        bd1=b2(inputs["bd1"], 4),
        bd2=b2(inputs["bd2"], 2),
        bdf=b2(inputs["bdf"], 1),
        bt1=b2(inputs["bt1"], 8),
        bt2=b2(inputs["bt2"], 8),
        bt3=b2(inputs["bt3"], 4),
        bt4=b2(inputs["bt4"], 2),
        bo=np.asarray(inputs["bo"], np.float32).reshape(1, 1),
        ident=np.eye(P, dtype=bf),
    )

    in_maps = []
    for c in range(N_CORES):
        sl = slice(c * cfg.bc, (c + 1) * cfg.bc)
        ci = cat[sl].astype(np.int64)
        rows = (np.arange(N_CAT, dtype=np.int64) * cfg.vocab)[None, :] + ci
        # idx[p, t*26+j] = row of sample (t*128+p), feature j
        idxc = (
            rows.reshape(cfg.nblk, P, N_CAT)
            .transpose(1, 0, 2)
            .reshape(P, cfg.nblk * N_CAT)
            .astype(np.int32)
        )
        xtc = np.ascontiguousarray(dx[sl].T.astype(bf))  # [13, bc]
        in_maps.append(dict(shared, idx=idxc, xt=xtc))
    return in_maps


def run_cores(inputs, cfg: Cfg, trace=False, **kw):
    import concourse.bass_utils as bass_utils

    nc = _get_nc(cfg)
    in_maps = _prep_host(inputs, cfg)
    res = bass_utils.run_bass_kernel_spmd(
        nc, in_maps, core_ids=list(range(N_CORES)), trace=trace, **kw
    )
    out = np.concatenate([r["out"] for r in res.results], axis=0)
    return out, res


def kernel(**inputs) -> np.ndarray:
    cfg = Cfg()
    out, _ = run_cores(inputs, cfg)
    return out.astype(np.float32)


# revision 8
# speedup vs baseline: 1.2956x; 1.2956x over previous
"""DLRM (embedding gather + bottom MLP + pairwise interactions + top MLP)
on 8 Trainium2 NeuronCores, data-parallel over the batch.

Sharding: batch 16384 -> 8 cores x 2048 samples. The embedding table
(26 x 100000 x 128 fp32, 1.33 GB) is replicated to every core's HBM, so no
collectives are needed; each core runs the full model on its batch slice.

Per-core device program:
  - indirect_dma_start gathers 26 rows/sample (512B descriptors) into
    [128 samples, 26*128] SBUF tiles, one DMA per 128-sample block.
  - DVE casts to bf16; PE transposes each [128,128] block to build
    ZT [E=128, 27 cols/sample] (26 features + bottom-MLP output).
  - Interactions: per-4-sample packed matmuls lhsT=rhs=ZT-slice
    [128, 4x27] -> S [108,108] PSUM (block-diagonal useful entries).
  - The strict-upper-triangle extraction is folded into the top-MLP first
    layer: W1full[27i+j] = 0.5*Wt1[pair(i,j)] (0 on diagonal), so layer 1
    consumes all 729 S values. S is routed through a DRAM round-trip
    ([sample, 729] rows) + PE transposes to get X1T [729, batch].
  - Top MLP 729(+128 dense)->1024->1024->512->256->1 in bf16 with fp32
    PSUM accumulation, transposed activations, sigmoid output.
"""

import numpy as np
import ml_dtypes

import concourse.bass as bass
import concourse.mybir as mybir
import concourse.tile as tile
from concourse import bacc
from concourse.bass import ds, ts

F32 = mybir.dt.float32
BF16 = mybir.dt.bfloat16
I32 = mybir.dt.int32

N_CAT = 26
N_FEAT = 27          # 26 embeddings + dense
E = 128
P = 128
N_CORES = 8

AFT = mybir.ActivationFunctionType


class Cfg:
    def __init__(self, vocab=100000, nblk=16):
        self.vocab = vocab
        self.nblk = nblk                  # 128-sample blocks per core
        self.bc = nblk * P                # batch per core
        self.vblk = min(4, nblk)          # blocks per btile
        self.vt = self.vblk * P           # btile size (free dim N)
        self.nv = nblk // self.vblk       # btiles per core


def build_kernel(nc: bass.Bass, tc: tile.TileContext, cfg: Cfg):
    from contextlib import ExitStack
    with ExitStack() as ctx:
        _build_kernel(nc, tc, cfg, ctx)


def _build_kernel(nc: bass.Bass, tc: tile.TileContext, cfg: Cfg, ctx):
    NBLK, BC, VT, NV, VBLK = cfg.nblk, cfg.bc, cfg.vt, cfg.nv, cfg.vblk

    # ---------------- DRAM I/O ----------------
    emb = nc.dram_tensor("emb", [N_CAT * cfg.vocab, E], BF16, kind="ExternalInput").ap()
    idx_d = nc.dram_tensor("idx", [P, NBLK * N_CAT], I32, kind="ExternalInput").ap()
    xt_d = nc.dram_tensor("xt", [13, BC], BF16, kind="ExternalInput").ap()

    def win(name, shape):
        return nc.dram_tensor(name, shape, BF16, kind="ExternalInput").ap()

    wd1_d = win("wd1", [13, 512])
    wd2_d = win("wd2", [512, 256])
    wdf_d = win("wdf", [256, 128])
    w1f_d = win("w1f", [729, 1024])
    w1d_d = win("w1d", [128, 1024])
    wt2_d = win("wt2", [1024, 1024])
    wt3_d = win("wt3", [1024, 512])
    wt4_d = win("wt4", [512, 256])
    wo_d = win("wo", [256, 1])

    def bin_(name, shape):
        return nc.dram_tensor(name, shape, F32, kind="ExternalInput").ap()

    bd1_d = bin_("bd1", [P, 4])
    bd2_d = bin_("bd2", [P, 2])
    bdf_d = bin_("bdf", [P, 1])
    bt1_d = bin_("bt1", [P, 8])
    bt2_d = bin_("bt2", [P, 8])
    bt3_d = bin_("bt3", [P, 4])
    bt4_d = bin_("bt4", [P, 2])
    bo_d = bin_("bo", [1, 1])
    ident_d = nc.dram_tensor("ident", [P, P], BF16, kind="ExternalInput").ap()

    out_d = nc.dram_tensor("out", [BC, 1], F32, kind="ExternalOutput").ap()

    # DRAM scratch for the S -> X1 relayout, one per 128-sample block for
    # fine-grained write->read dependencies.
    x1d = [
        nc.dram_tensor(f"x1scratch{t}", [P, 729], BF16, kind="Internal").ap()
        for t in range(NBLK)
    ]

    # ---------------- pools ----------------
    const = ctx.enter_context(tc.tile_pool(name="const", bufs=1))
    gpool = ctx.enter_context(tc.tile_pool(name="gather", bufs=2))
    zpool = ctx.enter_context(tc.tile_pool(name="zt", bufs=2))
    spool = ctx.enter_context(tc.tile_pool(name="spack", bufs=2))
    x1pool = ctx.enter_context(tc.tile_pool(name="x1", bufs=2))
    xtpool = ctx.enter_context(tc.tile_pool(name="x1t", bufs=2))
    opool = ctx.enter_context(tc.tile_pool(name="acts", bufs=1))
    p_t = ctx.enter_context(tc.tile_pool(name="ptrans", bufs=3, space="PSUM"))
    p_s = ctx.enter_context(tc.tile_pool(name="pint", bufs=2, space="PSUM"))
    p_m = ctx.enter_context(tc.tile_pool(name="pmlp", bufs=3, space="PSUM"))

    # ---------------- load constants ----------------
    idx_sb = const.tile([P, NBLK * N_CAT], I32)
    nc.sync.dma_start(out=idx_sb[:], in_=idx_d)
    ident = const.tile([P, P], BF16)
    nc.sync.dma_start(out=ident[:], in_=ident_d)

    def load_w(name, d, k, n):
        """Load [K, N] weight as list of [<=128, n] SBUF tiles."""
        tiles = []
        for i in range((k + P - 1) // P):
            ck = min(P, k - i * P)
            t = const.tile([ck, n], BF16, name=f"{name}_{i}")
            nc.sync.dma_start(out=t[:], in_=d[i * P : i * P + ck, :])
            tiles.append(t)
        return tiles

    wd1 = load_w("wd1", wd1_d, 13, 512)
    wd2 = load_w("wd2", wd2_d, 512, 256)
    wdf = load_w("wdf", wdf_d, 256, 128)
    w1f = load_w("w1f", w1f_d, 729, 1024)
    w1d = load_w("w1d", w1d_d, 128, 1024)
    wt2 = load_w("wt2", wt2_d, 1024, 1024)
    wt3 = load_w("wt3", wt3_d, 1024, 512)
    wt4 = load_w("wt4", wt4_d, 512, 256)
    wo = load_w("wo", wo_d, 256, 1)

    def load_b(name, d, nm):
        t = const.tile([d.shape[0], nm], F32, name=name)
        nc.sync.dma_start(out=t[:], in_=d)
        return t

    bd1 = load_b("bd1", bd1_d, 4)
    bd2 = load_b("bd2", bd2_d, 2)
    bdf = load_b("bdf", bdf_d, 1)
    bt1 = load_b("bt1", bt1_d, 8)
    bt2 = load_b("bt2", bt2_d, 8)
    bt3 = load_b("bt3", bt3_d, 4)
    bt4 = load_b("bt4", bt4_d, 2)
    bo = load_b("bo", bo_d, 1)

    # ---------------- bottom MLP (whole core batch) ----------------
    with tc.tile_pool(name="bottom", bufs=1) as bot:
        xtb = bot.tile([13, BC], BF16)
        nc.sync.dma_start(out=xtb[:], in_=xt_d)

        h1 = [bot.tile([P, BC], BF16, name=f"h1_{m}") for m in range(4)]
        for m in range(4):
            for v in range(NV):
                pm = p_m.tile([P, VT], F32)
                nc.tensor.matmul(
                    pm[:], wd1[0][:, ts(m, P)], xtb[:, ts(v, VT)],
                    start=True, stop=True,
                )
                nc.scalar.activation(
                    h1[m][:, ts(v, VT)], pm[:], AFT.Relu, bias=bd1[:, m : m + 1]
                )

        h2 = [bot.tile([P, BC], BF16, name=f"h2_{m}") for m in range(2)]
        for m in range(2):
            for v in range(NV):
                pm = p_m.tile([P, VT], F32)
                for k in range(4):
                    nc.tensor.matmul(
                        pm[:], wd2[k][:, ts(m, P)], h1[k][:, ts(v, VT)],
                        start=(k == 0), stop=(k == 3),
                    )
                nc.scalar.activation(
                    h2[m][:, ts(v, VT)], pm[:], AFT.Relu, bias=bd2[:, m : m + 1]
                )

        denseT = const.tile([P, BC], BF16)
        for v in range(NV):
            pm = p_m.tile([P, VT], F32)
            for k in range(2):
                nc.tensor.matmul(
                    pm[:], wdf[k][:], h2[k][:, ts(v, VT)],
                    start=(k == 0), stop=(k == 1),
                )
            nc.scalar.activation(
                denseT[:, ts(v, VT)], pm[:], AFT.Relu, bias=bdf[:, 0:1]
            )

    # ---------------- phase I: gather + transpose + interactions ----------------
    for t in range(NBLK):
        g = gpool.tile([P, N_CAT * E], BF16)
        nc.gpsimd.indirect_dma_start(
            out=g[:],
            out_offset=None,
            in_=emb,
            in_offset=bass.IndirectOffsetOnAxis(
                ap=idx_sb[:, t * N_CAT : (t + 1) * N_CAT], axis=0
            ),
        )

        # ZT: [E, (s, j)-major]; col s*27+j so 4-sample groups are
        # contiguous 108-col slices (walrus needs 1 free dim on rhs)
        zt = zpool.tile([P, P * N_FEAT], BF16)
        ztv = zt[:].rearrange("p (s j) -> p s j", j=N_FEAT)
        for j in range(N_CAT):
            pt = p_t.tile([P, P], BF16)
            nc.tensor.transpose(pt[:], g[:, ts(j, P)], ident[:])
            nc.any.tensor_copy(out=ztv[:, :, j], in_=pt[:])
        nc.vector.tensor_copy(
            out=ztv[:, :, N_CAT], in_=denseT[:, ts(t, P)]
        )

        # interactions: 32 packed matmuls of 4 samples each
        spk = spool.tile([108, 32 * 108], BF16)
        for q in range(32):
            pi = p_s.tile([108, 108], F32)
            op = zt[:, ds(4 * q * N_FEAT, 108)]
            nc.tensor.matmul(pi[:], op, op, start=True, stop=True)
            nc.any.tensor_copy(out=spk[:, ts(q, 108)], in_=pi[:])

        # write S -> x1 scratch rows [sample, 729]
        # spk row (s1*27+j1), col (q*108 + s2*27 + j2); take s1==s2==s
        for s in range(4):
            src = spk[ds(s * 27, 27), :].rearrange(
                "p (q s2 j2) -> p q s2 j2", s2=4, j2=27
            )[:, :, s, :]
            # dest row b = 4q + s ; iterate (j1, q, j2)
            dstv = x1d[t].rearrange(
                "(q s) (j1 j2) -> s j1 q j2", s=4, j2=27
            )[s, :, :, :]
            nc.sync.dma_start(out=dstv, in_=src)

    # ---------------- phase II: X1 relayout + top MLP per btile ----------------
    for v in range(NV):
        x1t = [xtpool.tile([min(P, 729 - k * P), VT], BF16, name=f"x1t_{k}")
               for k in range(6)]
        for tt in range(VBLK):
            x1s = x1pool.tile([P, 729], BF16)
            nc.sync.dma_start(out=x1s[:], in_=x1d[v * VBLK + tt][:])
            for k in range(6):
                ck = min(P, 729 - k * P)
                pt = p_t.tile([P, P], BF16)
                nc.tensor.transpose(pt[:ck, :], x1s[:, ds(k * P, ck)], ident[:])
                nc.any.tensor_copy(out=x1t[k][:, ts(tt, P)], in_=pt[:ck, :])

        # layer 1: K = 729 pairs + 128 dense
        o1 = opool.tile([P, 8 * VT], BF16, name="o1")
        for m in range(8):
            pm = p_m.tile([P, VT], F32)
            for k in range(6):
                nc.tensor.matmul(
                    pm[:], w1f[k][:, ts(m, P)], x1t[k][:],
                    start=(k == 0), stop=False,
                )
            nc.tensor.matmul(
                pm[:], w1d[0][:, ts(m, P)], denseT[:, ts(v, VT)],
                start=False, stop=True,
            )
            nc.scalar.activation(
                o1[:, ts(m, VT)], pm[:], AFT.Relu, bias=bt1[:, m : m + 1]
            )

        o2 = opool.tile([P, 8 * VT], BF16, name="o2")
        for m in range(8):
            pm = p_m.tile([P, VT], F32)
            for k in range(8):
                nc.tensor.matmul(
                    pm[:], wt2[k][:, ts(m, P)], o1[:, ts(k, VT)],
                    start=(k == 0), stop=(k == 7),
                )
            nc.scalar.activation(
                o2[:, ts(m, VT)], pm[:], AFT.Relu, bias=bt2[:, m : m + 1]
            )

        o3 = opool.tile([P, 4 * VT], BF16, name="o3")
        for m in range(4):
            pm = p_m.tile([P, VT], F32)
            for k in range(8):
                nc.tensor.matmul(
                    pm[:], wt3[k][:, ts(m, P)], o2[:, ts(k, VT)],
                    start=(k == 0), stop=(k == 7),
                )
            nc.scalar.activation(
                o3[:, ts(m, VT)], pm[:], AFT.Relu, bias=bt3[:, m : m + 1]
            )

        o4 = opool.tile([P, 2 * VT], BF16, name="o4")
        for m in range(2):
            pm = p_m.tile([P, VT], F32)
            for k in range(4):
                nc.tensor.matmul(
                    pm[:], wt4[k][:, ts(m, P)], o3[:, ts(k, VT)],
                    start=(k == 0), stop=(k == 3),
                )
            nc.scalar.activation(
                o4[:, ts(m, VT)], pm[:], AFT.Relu, bias=bt4[:, m : m + 1]
            )

        pm = p_m.tile([1, VT], F32)
        for k in range(2):
            nc.tensor.matmul(
                pm[:], wo[k][:], o4[:, ts(k, VT)],
                start=(k == 0), stop=(k == 1),
            )
        zf = x1pool.tile([1, VT], F32, name="zfinal")
        nc.scalar.activation(zf[:], pm[:], AFT.Sigmoid, bias=bo[:])
        nc.sync.dma_start(
            out=out_d[ts(v, VT), :].rearrange("b one -> one b"), in_=zf[:]
        )


# ---------------------------------------------------------------------------
# host side
# ---------------------------------------------------------------------------

_CACHE = {}


def _get_nc(cfg: Cfg):
    key = (cfg.vocab, cfg.nblk)
    if key in _CACHE:
        return _CACHE[key]
    nc = bacc.Bacc(
        "TRN2",
        target_bir_lowering=False,
        debug=False,
        enable_asserts=False,
        num_devices=N_CORES,
    )
    with tile.TileContext(nc) as tc:
        build_kernel(nc, tc, cfg)
    nc.compile()
    _CACHE[key] = nc
    return nc


def _prep_host(inputs, cfg: Cfg):
    """Build the per-core in_maps from full inputs."""
    bf = ml_dtypes.bfloat16
    emb = np.ascontiguousarray(
        np.asarray(inputs["emb_table"], dtype=bf).reshape(N_CAT * cfg.vocab, E)
    )
    cat = np.asarray(inputs["cat_idx"])
    dx = np.asarray(inputs["dense_x"], dtype=np.float32)

    iu, ju = np.triu_indices(N_FEAT, k=1)
    wt1 = np.asarray(inputs["Wt1"], dtype=np.float32)  # [479, 1024]
    w1f = np.zeros((N_FEAT, N_FEAT, 1024), dtype=np.float32)
    w1f[iu, ju] = 0.5 * wt1[: len(iu)]
    w1f[ju, iu] = 0.5 * wt1[: len(iu)]
    w1f = w1f.reshape(729, 1024)
    w1d = wt1[len(iu) :]  # [128, 1024]

    def b2(x, nm):  # bias [N] -> [128, nm]
        return np.ascontiguousarray(
            np.asarray(x, np.float32).reshape(nm, P).T
        )

    shared = dict(
        emb=emb,
        wd1=np.asarray(inputs["Wd1"], bf),
        wd2=np.asarray(inputs["Wd2"], bf),
        wdf=np.asarray(inputs["Wdf"], bf),
        w1f=np.asarray(w1f, bf),
        w1d=np.ascontiguousarray(np.asarray(w1d, bf)),
        wt2=np.asarray(inputs["Wt2"], bf),
        wt3=np.asarray(inputs["Wt3"], bf),
        wt4=np.asarray(inputs["Wt4"], bf),
        wo=np.asarray(inputs["Wo"], bf),
        bd1=b2(inputs["bd1"], 4),
        bd2=b2(inputs["bd2"], 2),
        bdf=b2(inputs["bdf"], 1),
        bt1=b2(inputs["bt1"], 8),
        bt2=b2(inputs["bt2"], 8),
        bt3=b2(inputs["bt3"], 4),
        bt4=b2(inputs["bt4"], 2),
        bo=np.asarray(inputs["bo"], np.float32).reshape(1, 1),
        ident=np.eye(P, dtype=bf),
    )

    in_maps = []
    for c in range(N_CORES):
        sl = slice(c * cfg.bc, (c + 1) * cfg.bc)
        ci = cat[sl].astype(np.int64)
        rows = (np.arange(N_CAT, dtype=np.int64) * cfg.vocab)[None, :] + ci
        # idx[p, t*26+j] = row of sample (t*128+p), feature j
        idxc = (
            rows.reshape(cfg.nblk, P, N_CAT)
            .transpose(1, 0, 2)
            .reshape(P, cfg.nblk * N_CAT)
            .astype(np.int32)
        )
        xtc = np.ascontiguousarray(dx[sl].T.astype(bf))  # [13, bc]
        in_maps.append(dict(shared, idx=idxc, xt=xtc))
    return in_maps


def run_cores(inputs, cfg: Cfg, trace=False, **kw):
    import concourse.bass_utils as bass_utils

    nc = _get_nc(cfg)
    in_maps = _prep_host(inputs, cfg)
    res = bass_utils.run_bass_kernel_spmd(
        nc, in_maps, core_ids=list(range(N_CORES)), trace=trace, **kw
    )
    out = np.concatenate([r["out"] for r in res.results], axis=0)
    return out, res


def kernel(**inputs) -> np.ndarray:
    cfg = Cfg()
    out, _ = run_cores(inputs, cfg)
    return out.astype(np.float32)


# revision 10
# speedup vs baseline: 1.3258x; 1.0233x over previous
"""DLRM (embedding gather + bottom MLP + pairwise interactions + top MLP)
on 8 Trainium2 NeuronCores, data-parallel over the batch.

Sharding: batch 16384 -> 8 cores x 2048 samples. The embedding table
(26 x 100000 x 128 fp32, 1.33 GB) is replicated to every core's HBM, so no
collectives are needed; each core runs the full model on its batch slice.

Per-core device program:
  - indirect_dma_start gathers 26 rows/sample (512B descriptors) into
    [128 samples, 26*128] SBUF tiles, one DMA per 128-sample block.
  - DVE casts to bf16; PE transposes each [128,128] block to build
    ZT [E=128, 27 cols/sample] (26 features + bottom-MLP output).
  - Interactions: per-4-sample packed matmuls lhsT=rhs=ZT-slice
    [128, 4x27] -> S [108,108] PSUM (block-diagonal useful entries).
  - The strict-upper-triangle extraction is folded into the top-MLP first
    layer: W1full[27i+j] = 0.5*Wt1[pair(i,j)] (0 on diagonal), so layer 1
    consumes all 729 S values. S is routed through a DRAM round-trip
    ([sample, 729] rows) + PE transposes to get X1T [729, batch].
  - Top MLP 729(+128 dense)->1024->1024->512->256->1 in bf16 with fp32
    PSUM accumulation, transposed activations, sigmoid output.
"""

import numpy as np
import ml_dtypes

import concourse.bass as bass
import concourse.mybir as mybir
import concourse.tile as tile
from concourse import bacc
from concourse.bass import ds, ts

F32 = mybir.dt.float32
BF16 = mybir.dt.bfloat16
I32 = mybir.dt.int32

N_CAT = 26
N_FEAT = 27          # 26 embeddings + dense
E = 128
P = 128
N_CORES = 8

AFT = mybir.ActivationFunctionType


class Cfg:
    def __init__(self, vocab=100000, nblk=16):
        self.vocab = vocab
        self.nblk = nblk                  # 128-sample blocks per core
        self.bc = nblk * P                # batch per core
        self.vblk = min(4, nblk)          # blocks per btile
        self.vt = self.vblk * P           # btile size (free dim N)
        self.nv = nblk // self.vblk       # btiles per core


def build_kernel(nc: bass.Bass, tc: tile.TileContext, cfg: Cfg):
    from contextlib import ExitStack
    with ExitStack() as ctx:
        _build_kernel(nc, tc, cfg, ctx)


def _build_kernel(nc: bass.Bass, tc: tile.TileContext, cfg: Cfg, ctx):
    NBLK, BC, VT, NV, VBLK = cfg.nblk, cfg.bc, cfg.vt, cfg.nv, cfg.vblk

    # ---------------- DRAM I/O ----------------
    emb = nc.dram_tensor("emb", [N_CAT * cfg.vocab, E], BF16, kind="ExternalInput").ap()
    idx_d = nc.dram_tensor("idx", [P, NBLK * N_CAT], I32, kind="ExternalInput").ap()
    xt_d = nc.dram_tensor("xt", [13, BC], BF16, kind="ExternalInput").ap()

    def win(name, shape):
        return nc.dram_tensor(name, shape, BF16, kind="ExternalInput").ap()

    wd1_d = win("wd1", [13, 512])
    wd2_d = win("wd2", [512, 256])
    wdf_d = win("wdf", [256, 128])
    w1f_d = win("w1f", [729, 1024])
    w1d_d = win("w1d", [128, 1024])
    wt2_d = win("wt2", [1024, 1024])
    wt3_d = win("wt3", [1024, 512])
    wt4_d = win("wt4", [512, 256])
    wo_d = win("wo", [256, 1])

    def bin_(name, shape):
        return nc.dram_tensor(name, shape, F32, kind="ExternalInput").ap()

    bd1_d = bin_("bd1", [P, 4])
    bd2_d = bin_("bd2", [P, 2])
    bdf_d = bin_("bdf", [P, 1])
    bt1_d = bin_("bt1", [P, 8])
    bt2_d = bin_("bt2", [P, 8])
    bt3_d = bin_("bt3", [P, 4])
    bt4_d = bin_("bt4", [P, 2])
    bo_d = bin_("bo", [1, 1])
    ident_d = nc.dram_tensor("ident", [P, P], BF16, kind="ExternalInput").ap()

    out_d = nc.dram_tensor("out", [BC, 1], F32, kind="ExternalOutput").ap()

    # DRAM scratch for the S -> X1 relayout, one per 128-sample block for
    # fine-grained write->read dependencies.
    x1d = [
        nc.dram_tensor(f"x1scratch{t}", [P, 729], BF16, kind="Internal").ap()
        for t in range(NBLK)
    ]

    # ---------------- pools ----------------
    const = ctx.enter_context(tc.tile_pool(name="const", bufs=1))
    gpool = ctx.enter_context(tc.tile_pool(name="gather", bufs=2))
    zpool = ctx.enter_context(tc.tile_pool(name="zt", bufs=2))
    spool = ctx.enter_context(tc.tile_pool(name="spack", bufs=2))
    x1pool = ctx.enter_context(tc.tile_pool(name="x1", bufs=2))
    xtpool = ctx.enter_context(tc.tile_pool(name="x1t", bufs=2))
    opool = ctx.enter_context(tc.tile_pool(name="acts", bufs=1))
    p_t = ctx.enter_context(tc.tile_pool(name="ptrans", bufs=3, space="PSUM"))
    p_s = ctx.enter_context(tc.tile_pool(name="pint", bufs=2, space="PSUM"))
    p_m = ctx.enter_context(tc.tile_pool(name="pmlp", bufs=3, space="PSUM"))

    # ---------------- load constants ----------------
    idx_sb = const.tile([P, NBLK * N_CAT], I32)
    nc.sync.dma_start(out=idx_sb[:], in_=idx_d)
    ident = const.tile([P, P], BF16)
    nc.sync.dma_start(out=ident[:], in_=ident_d)

    def load_w(name, d, k, n):
        """Load [K, N] weight as list of [<=128, n] SBUF tiles."""
        tiles = []
        for i in range((k + P - 1) // P):
            ck = min(P, k - i * P)
            t = const.tile([ck, n], BF16, name=f"{name}_{i}")
            nc.sync.dma_start(out=t[:], in_=d[i * P : i * P + ck, :])
            tiles.append(t)
        return tiles

    wd1 = load_w("wd1", wd1_d, 13, 512)
    wd2 = load_w("wd2", wd2_d, 512, 256)
    wdf = load_w("wdf", wdf_d, 256, 128)
    w1f = load_w("w1f", w1f_d, 729, 1024)
    w1d = load_w("w1d", w1d_d, 128, 1024)
    wt2 = load_w("wt2", wt2_d, 1024, 1024)
    wt3 = load_w("wt3", wt3_d, 1024, 512)
    wt4 = load_w("wt4", wt4_d, 512, 256)
    wo = load_w("wo", wo_d, 256, 1)

    def load_b(name, d, nm):
        t = const.tile([d.shape[0], nm], F32, name=name)
        nc.sync.dma_start(out=t[:], in_=d)
        return t

    bd1 = load_b("bd1", bd1_d, 4)
    bd2 = load_b("bd2", bd2_d, 2)
    bdf = load_b("bdf", bdf_d, 1)
    bt1 = load_b("bt1", bt1_d, 8)
    bt2 = load_b("bt2", bt2_d, 8)
    bt3 = load_b("bt3", bt3_d, 4)
    bt4 = load_b("bt4", bt4_d, 2)
    bo = load_b("bo", bo_d, 1)

    # ---------------- bottom MLP (whole core batch) ----------------
    with tc.tile_pool(name="bottom", bufs=1) as bot:
        xtb = bot.tile([13, BC], BF16)
        nc.sync.dma_start(out=xtb[:], in_=xt_d)

        h1 = [bot.tile([P, BC], BF16, name=f"h1_{m}") for m in range(4)]
        for m in range(4):
            for v in range(NV):
                pm = p_m.tile([P, VT], F32)
                nc.tensor.matmul(
                    pm[:], wd1[0][:, ts(m, P)], xtb[:, ts(v, VT)],
                    start=True, stop=True,
                )
                nc.scalar.activation(
                    h1[m][:, ts(v, VT)], pm[:], AFT.Relu, bias=bd1[:, m : m + 1]
                )

        h2 = [bot.tile([P, BC], BF16, name=f"h2_{m}") for m in range(2)]
        for m in range(2):
            for v in range(NV):
                pm = p_m.tile([P, VT], F32)
                for k in range(4):
                    nc.tensor.matmul(
                        pm[:], wd2[k][:, ts(m, P)], h1[k][:, ts(v, VT)],
                        start=(k == 0), stop=(k == 3),
                    )
                nc.scalar.activation(
                    h2[m][:, ts(v, VT)], pm[:], AFT.Relu, bias=bd2[:, m : m + 1]
                )

        denseT = const.tile([P, BC], BF16)
        for v in range(NV):
            pm = p_m.tile([P, VT], F32)
            for k in range(2):
                nc.tensor.matmul(
                    pm[:], wdf[k][:], h2[k][:, ts(v, VT)],
                    start=(k == 0), stop=(k == 1),
                )
            nc.scalar.activation(
                denseT[:, ts(v, VT)], pm[:], AFT.Relu, bias=bdf[:, 0:1]
            )

    # ---------------- phase I: gather + transpose + interactions ----------------
    for t in range(NBLK):
        g = gpool.tile([P, N_CAT * E], BF16)
        nc.gpsimd.indirect_dma_start(
            out=g[:],
            out_offset=None,
            in_=emb,
            in_offset=bass.IndirectOffsetOnAxis(
                ap=idx_sb[:, t * N_CAT : (t + 1) * N_CAT], axis=0
            ),
        )

        # ZT: [E, (s, j)-major]; col s*27+j so 4-sample groups are
        # contiguous 108-col slices (walrus needs 1 free dim on rhs)
        zt = zpool.tile([P, P * N_FEAT], BF16)
        ztv = zt[:].rearrange("p (s j) -> p s j", j=N_FEAT)
        for j0 in range(0, N_CAT, 4):
            nj = min(4, N_CAT - j0)
            pt = p_t.tile([P, 4 * P], BF16)
            for j in range(j0, j0 + nj):
                nc.tensor.transpose(
                    pt[:, ts(j - j0, P)], g[:, ts(j, P)], ident[:]
                )
            # psum cols (j, s) -> zt cols s*27+j : dest dims (j outer, s inner)
            dst = ztv[:, :, j0 : j0 + nj].rearrange("p s j -> p j s")
            nc.any.tensor_copy(out=dst, in_=pt[:, : nj * P])
        nc.vector.tensor_copy(
            out=ztv[:, :, N_CAT], in_=denseT[:, ts(t, P)]
        )

        # interactions: 32 packed matmuls of 4 samples each
        spk = spool.tile([108, 32 * 108], BF16)
        for q0 in range(0, 32, 4):
            pi = p_s.tile([108, 4 * 108], F32)
            for q in range(q0, q0 + 4):
                op = zt[:, ds(4 * q * N_FEAT, 108)]
                nc.tensor.matmul(
                    pi[:, ts(q - q0, 108)], op, op, start=True, stop=True
                )
            nc.any.tensor_copy(out=spk[:, ds(q0 * 108, 4 * 108)], in_=pi[:])

        # write S -> x1 scratch rows [sample, 729]
        # spk row (s1*27+j1), col (q*108 + s2*27 + j2); take s1==s2==s
        for s in range(4):
            src = spk[ds(s * 27, 27), :].rearrange(
                "p (q s2 j2) -> p q s2 j2", s2=4, j2=27
            )[:, :, s, :]
            # dest row b = 4q + s ; iterate (j1, q, j2)
            dstv = x1d[t].rearrange(
                "(q s) (j1 j2) -> s j1 q j2", s=4, j2=27
            )[s, :, :, :]
            nc.sync.dma_start(out=dstv, in_=src)

    # ---------------- phase II: X1 relayout + top MLP per btile ----------------
    for v in range(NV):
        x1ta = xtpool.tile([P, 6 * VT], BF16, name="x1ta")
        x1t = [x1ta[:, ts(k, VT)] for k in range(6)]
        for tt in range(VBLK):
            x1s = x1pool.tile([P, 768], BF16)
            nc.any.memset(x1s[:, 729:768], 0.0)
            nc.sync.dma_start(out=x1s[:, :729], in_=x1d[v * VBLK + tt][:])
            pt = p_t.tile([P, 6 * P], BF16)
            for k in range(6):
                nc.tensor.transpose(
                    pt[:, ts(k, P)], x1s[:, ts(k, P)], ident[:]
                )
            # psum cols (k, 128) -> x1ta cols (k, tt*128+128)
            dst = x1ta[:].rearrange("p (k b) -> p k b", k=6)[
                :, :, ts(tt, P)
            ]
            nc.any.tensor_copy(out=dst, in_=pt[:])
        x1t = [x1t[k] for k in range(6)]

        # layer 1: K = 729 pairs + 128 dense
        o1 = opool.tile([P, 8 * VT], BF16, name="o1")
        for m in range(8):
            pm = p_m.tile([P, VT], F32)
            for k in range(6):
                nc.tensor.matmul(
                    pm[:], w1f[k][:, ts(m, P)], x1t[k][: w1f[k].shape[0], :],
                    start=(k == 0), stop=False,
                )
            nc.tensor.matmul(
                pm[:], w1d[0][:, ts(m, P)], denseT[:, ts(v, VT)],
                start=False, stop=True,
            )
            nc.scalar.activation(
                o1[:, ts(m, VT)], pm[:], AFT.Relu, bias=bt1[:, m : m + 1]
            )

        o2 = opool.tile([P, 8 * VT], BF16, name="o2")
        for m in range(8):
            pm = p_m.tile([P, VT], F32)
            for k in range(8):
                nc.tensor.matmul(
                    pm[:], wt2[k][:, ts(m, P)], o1[:, ts(k, VT)],
                    start=(k == 0), stop=(k == 7),
                )
            nc.scalar.activation(
                o2[:, ts(m, VT)], pm[:], AFT.Relu, bias=bt2[:, m : m + 1]
            )

        o3 = opool.tile([P, 4 * VT], BF16, name="o3")
        for m in range(4):
            pm = p_m.tile([P, VT], F32)
            for k in range(8):
                nc.tensor.matmul(
                    pm[:], wt3[k][:, ts(m, P)], o2[:, ts(k, VT)],
                    start=(k == 0), stop=(k == 7),
                )
            nc.scalar.activation(
                o3[:, ts(m, VT)], pm[:], AFT.Relu, bias=bt3[:, m : m + 1]
            )

        o4 = opool.tile([P, 2 * VT], BF16, name="o4")
        for m in range(2):
            pm = p_m.tile([P, VT], F32)
            for k in range(4):
                nc.tensor.matmul(
                    pm[:], wt4[k][:, ts(m, P)], o3[:, ts(k, VT)],
                    start=(k == 0), stop=(k == 3),
                )
            nc.scalar.activation(
                o4[:, ts(m, VT)], pm[:], AFT.Relu, bias=bt4[:, m : m + 1]
            )

        pm = p_m.tile([1, VT], F32)
        for k in range(2):
            nc.tensor.matmul(
                pm[:], wo[k][:], o4[:, ts(k, VT)],
                start=(k == 0), stop=(k == 1),
            )
        zf = x1pool.tile([1, VT], F32, name="zfinal")
        nc.scalar.activation(zf[:], pm[:], AFT.Sigmoid, bias=bo[:])
        nc.sync.dma_start(
            out=out_d[ts(v, VT), :].rearrange("b one -> one b"), in_=zf[:]
        )


# ---------------------------------------------------------------------------
# host side
# ---------------------------------------------------------------------------

_CACHE = {}


def _get_nc(cfg: Cfg):
    key = (cfg.vocab, cfg.nblk)
    if key in _CACHE:
        return _CACHE[key]
    nc = bacc.Bacc(
        "TRN2",
        target_bir_lowering=False,
        debug=False,
        enable_asserts=False,
        num_devices=N_CORES,
    )
    with tile.TileContext(nc) as tc:
        build_kernel(nc, tc, cfg)
    nc.compile()
    _CACHE[key] = nc
    return nc


def _prep_host(inputs, cfg: Cfg):
    """Build the per-core in_maps from full inputs."""
    bf = ml_dtypes.bfloat16
    emb = np.ascontiguousarray(
        np.asarray(inputs["emb_table"], dtype=bf).reshape(N_CAT * cfg.vocab, E)
    )
    cat = np.asarray(inputs["cat_idx"])
    dx = np.asarray(inputs["dense_x"], dtype=np.float32)

    iu, ju = np.triu_indices(N_FEAT, k=1)
    wt1 = np.asarray(inputs["Wt1"], dtype=np.float32)  # [479, 1024]
    w1f = np.zeros((N_FEAT, N_FEAT, 1024), dtype=np.float32)
    w1f[iu, ju] = 0.5 * wt1[: len(iu)]
    w1f[ju, iu] = 0.5 * wt1[: len(iu)]
    w1f = w1f.reshape(729, 1024)
    w1d = wt1[len(iu) :]  # [128, 1024]

    def b2(x, nm):  # bias [N] -> [128, nm]
        return np.ascontiguousarray(
            np.asarray(x, np.float32).reshape(nm, P).T
        )

    shared = dict(
        emb=emb,
        wd1=np.asarray(inputs["Wd1"], bf),
        wd2=np.asarray(inputs["Wd2"], bf),
        wdf=np.asarray(inputs["Wdf"], bf),
        w1f=np.asarray(w1f, bf),
        w1d=np.ascontiguousarray(np.asarray(w1d, bf)),
        wt2=np.asarray(inputs["Wt2"], bf),
        wt3=np.asarray(inputs["Wt3"], bf),
        wt4=np.asarray(inputs["Wt4"], bf),
        wo=np.asarray(inputs["Wo"], bf),
        bd1=b2(inputs["bd1"], 4),
        bd2=b2(inputs["bd2"], 2),
        bdf=b2(inputs["bdf"], 1),
        bt1=b2(inputs["bt1"], 8),
        bt2=b2(inputs["bt2"], 8),
        bt3=b2(inputs["bt3"], 4),
        bt4=b2(inputs["bt4"], 2),
        bo=np.asarray(inputs["bo"], np.float32).reshape(1, 1),
        ident=np.eye(P, dtype=bf),
    )

    in_maps = []
    for c in range(N_CORES):
        sl = slice(c * cfg.bc, (c + 1) * cfg.bc)
        ci = cat[sl].astype(np.int64)
        rows = (np.arange(N_CAT, dtype=np.int64) * cfg.vocab)[None, :] + ci
        # idx[p, t*26+j] = row of sample (t*128+p), feature j
        idxc = (
            rows.reshape(cfg.nblk, P, N_CAT)
            .transpose(1, 0, 2)
            .reshape(P, cfg.nblk * N_CAT)
            .astype(np.int32)
        )
        xtc = np.ascontiguousarray(dx[sl].T.astype(bf))  # [13, bc]
        in_maps.append(dict(shared, idx=idxc, xt=xtc))
    return in_maps


def run_cores(inputs, cfg: Cfg, trace=False, **kw):
    import concourse.bass_utils as bass_utils

    nc = _get_nc(cfg)
    in_maps = _prep_host(inputs, cfg)
    res = bass_utils.run_bass_kernel_spmd(
        nc, in_maps, core_ids=list(range(N_CORES)), trace=trace, **kw
    )
    out = np.concatenate([r["out"] for r in res.results], axis=0)
    return out, res


def kernel(**inputs) -> np.ndarray:
    cfg = Cfg()
    out, _ = run_cores(inputs, cfg)
    return out.astype(np.float32)
